# revision 1
# baseline (speedup 1.0000x reference)
"""Boundary rendering module for Trainium2 (8 NeuronCores).

Computes, for x of shape (2, 4, 64, 256, 256) f32:
    mn/mx  = per-channel global min/max
    binary = ((x - mn) / (mx - mn)) > 0.5     [== (x - mn) > 0.5*(mx - mn)]
    dilated = 3x3x3 binary dilation of binary (SAME padding)
    out    = dilated - binary

Sharding: H (=256) split into 8 chunks of 32 rows, one per NeuronCore.
Each core receives its 32 rows plus one halo row on each side (global
edges padded with -1e30 so the halo mask is 0).  On-core layout puts
(B, D) = 128 on the SBUF partition axis; (C, H, W) live on the free axis.

Per-channel min/max: per-partition partials on DVE, transposed across
partitions with a tiny PE matmul against an identity, reduced on DVE,
then an 8-core AllReduce(max) collective over the pair (mx, -mn), and
broadcast back to 128 partitions with a rank-1 PE matmul.

The 3x3x3 dilation is computed as a count:
    count[p, h, w] = sum_{dw in {-1,0,1}} sum_q bandA[p,q] * mH[q, h, w+dw]
where mH is the H-dilated binary mask (2 vector max ops) and bandA is the
(b,d)-banded 0/1 matrix (D-axis window).  The W shifts are plain +-1 column
offsets into a zero-padded mask buffer, accumulated in PSUM by TensorE.
A final accumulating matmul adds -16 * binary, so
    psum >= 1  iff  count >= 1 and binary == 0
which a single saturated sigmoid activation turns into exact {0.0, 1.0}.
"""

import os
import sys

import numpy as np

for _p in ("/opt/trn_rl_repo", "/root/.axon_site/_ro/trn_rl_repo"):
    if os.path.isdir(_p) and _p not in sys.path:
        sys.path.insert(0, _p)

import ml_dtypes

B, C, D, H, W = 2, 4, 64, 256, 256
NCORES = 8
HS = H // NCORES  # 32 own rows per core
HA = HS + 2  # rows incl halo
HPAD = np.float32(-1e30)  # halo pad at global H edges -> mask 0

MHW = 258  # mH row width: 256 data cols + 2 zero pad cols
MHLEN = 33 * MHW + 2  # 33 rows (1 pad + 32 data) + slack for dw=+1 AP views

_CACHE = {}


def _consts():
    bd = np.arange(128)
    b = bd // D
    d = bd % D
    A = (b[:, None] == b[None, :]) & (np.abs(d[:, None] - d[None, :]) <= 1)
    A = A.astype(ml_dtypes.bfloat16)
    negI = (-16.0 * np.eye(128)).astype(ml_dtypes.bfloat16)
    I128 = np.eye(128, dtype=np.float32)
    return A, negI, I128


def _build(reps: int = 1, phase: str = "B", parts: str = "all"):
    import concourse.bass as bass
    import concourse.bacc as bacc
    import concourse.mybir as mybir
    import concourse.tile as tile
    from contextlib import ExitStack

    f32 = mybir.dt.float32
    bf16 = mybir.dt.bfloat16
    Alu = mybir.AluOpType

    nc = bacc.Bacc(
        "TRN2",
        target_bir_lowering=False,
        debug=False,
        num_devices=NCORES,
    )

    xs = nc.dram_tensor("xs", [B, C, D, HA, W], f32, kind="ExternalInput")
    if phase == "A":
        pmm = nc.dram_tensor("pmm", [8, 1], f32, kind="ExternalOutput")
        out = pm64 = None
    else:
        pm64 = nc.dram_tensor("pm64", [NCORES, 8], f32, kind="ExternalInput")
        out = nc.dram_tensor("out", [B, C, D, HS, W], f32, kind="ExternalOutput")
        pmm = None
    A_np, negI_np, I_np = _consts()
    bandA_d = nc.inline_tensor(A_np, name="bandA")
    negI_d = nc.inline_tensor(negI_np, name="negI")
    ident_d = nc.inline_tensor(I_np, name="ident")

    # partition axis = (b, d) = 128; DRAM-side APs keep b and d as separate
    # leading dims (DMA pairs elements in iteration order, b-major then d,
    # matching partition index p = b*64 + d).
    xsa = xs.ap()
    outa = out.ap() if out is not None else None

    with ExitStack() as ctx:
        tc = ctx.enter_context(tile.TileContext(nc))
        pers = ctx.enter_context(tc.tile_pool(name="pers", bufs=1))
        psump = ctx.enter_context(tc.tile_pool(name="psum", bufs=2, space="PSUM"))

        x_all = pers.tile([128, C, HA, W], f32)  # 136 KiB / partition
        binm = pers.tile([128, HA, W], bf16)  # 17 KiB  {0,1}
        mH = pers.tile([128, MHLEN], bf16)  # ~16.7 KiB  H-dilated mask
        stag = pers.tile([128, 4096], f32)  # out staging (16 rows)
        pmax = pers.tile([128, 16], f32)
        pmin = pers.tile([128, 16], f32)
        red8 = pers.tile([128, 8], f32)  # [mx(4) | -mn(4)] local
        s8 = pers.tile([128, 1], f32)  # per-partition reduced (parts 0..7)
        s1v = pers.tile([128, 72], f32)  # gathered partials + reduced vals
        gv8 = pers.tile([128, 8], f32)  # broadcast [mx | -mn] on all parts
        mnv = pers.tile([128, 4], f32)  # mn per channel
        h4 = pers.tile([128, 4], f32)  # 0.5*(mx-mn) per channel
        At = pers.tile([128, 128], bf16)
        Nt = pers.tile([128, 128], bf16)
        It = pers.tile([128, 128], f32)
        ones1 = pers.tile([128, 128], f32)  # row 0 used as all-ones lhsT
        sel_bias = pers.tile([128, 1], f32)

        nc.vector.memset(sel_bias[:, :], -100.0)
        nc.vector.memset(ones1[:, :], 1.0)
        nc.gpsimd.dma_start(out=At[:, :], in_=bandA_d.ap())
        nc.gpsimd.dma_start(out=Nt[:, :], in_=negI_d.ap())
        nc.gpsimd.dma_start(out=It[:, :], in_=ident_d.ap())
        nc.vector.memset(mH[:, :], 0.0)  # zero pads once; data rows rewritten
        if parts != "all":
            # stage-isolated benchmark builds: pre-write every cross-stage
            # buffer once so skipped producers don't leave unwritten reads
            nc.vector.memset(x_all[:, :, :, :], 0.0)
            nc.vector.memset(binm[:, :, :], 0.0)
            nc.vector.memset(stag[:, :], 0.0)
            nc.vector.memset(s8[:, :], 0.0)
            nc.vector.memset(s1v[:, :], 0.0)
            nc.vector.memset(red8[:, :], 0.0)
            nc.vector.memset(pmax[:, :], 0.0)
            nc.vector.memset(pmin[:, :], 0.0)

        for _rep in range(reps):
            if phase == "A":
                _pass_a(
                    nc, mybir, Alu, psump, xsa, pmm,
                    x_all, pmax, pmin, red8, s8, It, parts,
                )
            else:
                _pass_b(
                    nc, mybir, Alu, psump, xsa, outa, pm64,
                    x_all, binm, mH, stag, s1v, gv8,
                    mnv, h4, At, Nt, ones1, sel_bias, parts,
                )

    nc.compile()
    return nc


def _load_x(nc, xsa, x_all, parts):
    engines = [nc.sync, nc.scalar]
    for i in range(8):
        c, half = i // 2, i % 2
        eng = engines[i % len(engines)]
        eng.dma_start(
            out=x_all[:, c, 17 * half : 17 * half + 17, :],
            in_=xsa[:, c, :, 17 * half : 17 * half + 17, :],
        )


def _pass_a(
    nc, mybir, Alu, psump, xsa, pmm,
    x_all, pmax, pmin, red8, s8, It, parts="all",
):
    """Load the shard and reduce it to [mx(4) | -mn(4)] -> DRAM pmm[8,1]."""
    f32 = mybir.dt.float32
    on = lambda p: parts == "all" or p in parts
    if on("dma"):
        _load_x(nc, xsa, x_all, parts)
    else:
        # lite loads: defeat cross-rep CSE/DCE while costing ~nothing
        for c in range(C):
            nc.sync.dma_start(out=x_all[:, c, 0, :], in_=xsa[:, c, :, 0, :])
    if not on("dve"):
        nc.sync.dma_start(out=pmm.ap(), in_=s8[0:8, 0:1])
        return
    for c in range(C):
        for k in range(4):
            chunk = x_all[:, c, 1 + 8 * k : 9 + 8 * k, :]
            nc.vector.tensor_reduce(
                out=pmax[:, 4 * c + k : 4 * c + k + 1],
                in_=chunk,
                axis=mybir.AxisListType.XY,
                op=Alu.max,
            )
            nc.vector.tensor_reduce(
                out=pmin[:, 4 * c + k : 4 * c + k + 1],
                in_=chunk,
                axis=mybir.AxisListType.XY,
                op=Alu.min,
            )
    for c in range(C):
        nc.vector.tensor_reduce(
            out=red8[:, c : c + 1],
            in_=pmax[:, 4 * c : 4 * c + 4],
            axis=mybir.AxisListType.X,
            op=Alu.max,
        )
        nc.vector.tensor_reduce(
            out=red8[:, 4 + c : 5 + c],
            in_=pmin[:, 4 * c : 4 * c + 4],
            axis=mybir.AxisListType.X,
            op=Alu.min,
        )
    # negate the mins so a single max combines both downstream
    nc.vector.tensor_scalar_mul(red8[:, 4:8], red8[:, 4:8], -1.0)
    # cross-partition max: transpose red8 [128p, 8] -> psum [8p, 128] with a
    # PE matmul against the identity, then free-axis reduce on DVE.
    pst = psump.tile([128, 2048], f32, tag="ps")
    nc.tensor.matmul(pst[0:8, 0:128], red8[:, :], It[:, :], start=True, stop=True)
    nc.vector.tensor_reduce(
        out=s8[0:8, 0:1],
        in_=pst[0:8, 0:128],
        axis=mybir.AxisListType.X,
        op=Alu.max,
    )
    nc.sync.dma_start(out=pmm.ap(), in_=s8[0:8, 0:1])


def _pass_b(
    nc, mybir, Alu, psump, xsa, outa, pm64,
    x_all, binm, mH, stag, s1v, gv8,
    mnv, h4, At, Nt, ones1, sel_bias, parts="all",
):
    """Main pipeline: thresholds from pm64, mask, dilate, boundary."""
    f32 = mybir.dt.float32
    on = lambda p: parts == "all" or p in parts
    if on("dma"):
        for c in range(C):
            nc.sync.dma_start(out=x_all[:, c, :, :], in_=xsa[:, c, :, :, :])
    else:
        for c in range(C):
            nc.sync.dma_start(out=x_all[:, c, 0, :], in_=xsa[:, c, :, 0, :])

    # reduce the gathered per-core partials [8 cores, 8] over cores on
    # partition 0, then broadcast to all partitions with a rank-1 matmul.
    nc.sync.dma_start(out=s1v[0:1, 0:64], in_=pm64.ap().rearrange("k j -> (k j)")[None, :])
    nc.vector.tensor_reduce(
        out=s1v[0:1, 64:72],
        in_=s1v[0:1, 0:64].rearrange("p (k j) -> p j k", k=NCORES),
        axis=mybir.AxisListType.X,
        op=Alu.max,
    )
    psb = psump.tile([128, 2048], f32, tag="ps")
    nc.tensor.matmul(psb[:, 0:8], ones1[0:1, :], s1v[0:1, 64:72], start=True, stop=True)
    nc.vector.tensor_copy(gv8[:, :], psb[:, 0:8])
    nc.vector.tensor_scalar_mul(mnv[:, :], gv8[:, 4:8], -1.0)
    nc.vector.tensor_add(h4[:, :], gv8[:, 0:4], gv8[:, 4:8])
    nc.vector.tensor_scalar_mul(h4[:, :], h4[:, :], 0.5)

    # ---- mask, dilate, boundary ----
    mHd = mH[:, MHW : MHW + 32 * MHW].rearrange("p (r z) -> p r z", z=MHW)[
        :, :, 0:W
    ]
    for c in range(C):
        if on("dve"):
            nc.vector.tensor_scalar(
            out=binm[:, :, :],
            in0=x_all[:, c, :, :],
            scalar1=mnv[:, c : c + 1],
            scalar2=h4[:, c : c + 1],
            op0=Alu.subtract,
            op1=Alu.is_gt,
            )
            nc.vector.tensor_tensor(
                out=mHd,
                in0=binm[:, 0:HS, :],
                in1=binm[:, 2 : HS + 2, :],
                op=Alu.max,
            )
            nc.vector.tensor_tensor(
                out=mHd,
                in0=mHd,
                in1=binm[:, 1 : HS + 1, :],
                op=Alu.max,
            )
            if not on("pe"):
                # tiny live consumer of mH so DCE keeps the masks
                nc.vector.tensor_reduce(
                    out=stag[:, c : c + 1],
                    in_=mH[:, 0:128],
                    axis=mybir.AxisListType.X,
                    op=Alu.max,
                )
        for t in range(2):  # 16 own rows per staging buffer
            ps = psump.tile([128, 2048], f32, tag="ps")
            ps2 = psump.tile([128, 2048], f32, tag="ps")
            for half, pst_ in ((0, ps), (1, ps2)):
                if on("pe"):
                    for s in range(4):  # one PSUM bank = 2 rows = 512
                        R = 16 * t + 8 * half + 2 * s
                        pslice = pst_[:, 512 * s : 512 * s + 512]
                        for j, dw in enumerate((-1, 0, 1)):
                            off = (R + 1) * MHW + dw
                            rhs = mH[:, off : off + 2 * MHW].rearrange(
                                "p (r z) -> p r z", z=MHW
                            )[:, :, 0:W]
                            nc.tensor.matmul(
                                pslice,
                                At[:, :],
                                rhs,
                                start=(j == 0),
                                stop=False,
                            )
                        nc.tensor.matmul(
                            pslice,
                            Nt[:, :],
                            binm[:, 1 + R : 3 + R, :],
                            start=False,
                            stop=True,
                        )
                if on("pe") and not on("act"):
                    nc.vector.tensor_copy(
                        out=stag[:, 4 + 2 * half : 5 + 2 * half], in_=pst_[:, 0:1]
                    )
                if on("act"):
                    nc.scalar.activation(
                        out=stag[:, 2048 * half : 2048 * half + 2048],
                        in_=pst_[:, :],
                        func=mybir.ActivationFunctionType.Sigmoid,
                        bias=sel_bias[:, :],
                        scale=200.0,
                    )
            if on("store"):
                eng = nc.sync if (2 * c + t) % 2 == 0 else nc.scalar
                eng.dma_start(
                    out=outa[:, c, :, 16 * t : 16 * t + 16, :],
                    in_=stag.rearrange("p (r w) -> p r w", w=W),
                )


def _get_nc(phase="B"):
    key = "nc" + phase
    if key not in _CACHE:
        _CACHE[key] = _build(phase=phase)
    return _CACHE[key]


def _make_in_maps(x: np.ndarray):
    in_maps = []
    for k in range(NCORES):
        xs = np.empty((B, C, D, HA, W), np.float32)
        lo = k * HS
        xs[:, :, :, 1 : HS + 1, :] = x[:, :, :, lo : lo + HS, :]
        if k > 0:
            xs[:, :, :, 0, :] = x[:, :, :, lo - 1, :]
        else:
            xs[:, :, :, 0, :] = HPAD
        if k < NCORES - 1:
            xs[:, :, :, HS + 1, :] = x[:, :, :, lo + HS, :]
        else:
            xs[:, :, :, HS + 1, :] = HPAD
        in_maps.append({"xs": xs})
    return in_maps


def kernel(x: np.ndarray) -> np.ndarray:
    from concourse.bass_utils import run_bass_kernel_spmd

    x = np.ascontiguousarray(np.asarray(x), dtype=np.float32)
    assert x.shape == (B, C, D, H, W)

    in_maps = _make_in_maps(x)
    cores = list(range(NCORES))

    # launch A: per-core min/max partials
    res_a = run_bass_kernel_spmd(_get_nc("A"), in_maps, core_ids=cores)
    pm64 = np.concatenate(
        [res_a.results[k]["pmm"].reshape(1, 8) for k in range(NCORES)], axis=0
    )

    # launch B: full pipeline with the gathered partials
    in_maps_b = [{"xs": m["xs"], "pm64": pm64} for m in in_maps]
    res = run_bass_kernel_spmd(_get_nc("B"), in_maps_b, core_ids=cores)
    pieces = [res.results[k]["out"] for k in range(NCORES)]
    return np.concatenate(pieces, axis=3)


if __name__ == "__main__":
    x = np.random.randn(B, C, D, H, W).astype(np.float32)
    y = kernel(x)
    print(y.shape, y.dtype, y.sum())



# revision 2
# speedup vs baseline: 1.8796x; 1.8796x over previous
"""Boundary rendering module for Trainium2 (8 NeuronCores), fused single launch.

Computes, for x of shape (2, 4, 64, 256, 256) f32:
    mn/mx  = per-channel global min/max
    binary = ((x - mn) / (mx - mn)) > 0.5     [== (x - mn) > 0.5*(mx - mn)]
    dilated = 3x3x3 binary dilation of binary (SAME padding)
    out    = dilated - binary

Sharding: H (=256) split into 8 chunks of 32 rows, one per NeuronCore.
Each core receives its 32 rows plus one halo row on each side (global
edges padded with -1e30 so the halo mask is 0).  On-core layout puts
(B, D) = 128 on the SBUF partition axis; (C, H, W) live on the free axis.

Single launch per core:
  1. x loaded in 8 (channel, half) chunks via gpsimd SWDGE DMA (spreads
     across all 16 SDMA engines; the 2 HWDGE rings top out at ~75 GB/s).
  2. DVE min/max partials per chunk as each load lands (overlapped).
  3. Partials transposed across partitions with a PE identity matmul,
     reduced, then an 8-core AllReduce(max) over [mx(4) | -mn(4)]
     through DRAM bounce buffers; result broadcast back to all 128
     partitions with a rank-1 PE matmul.
  4. Mask + H-dilation on DVE (bf16), W+D dilation as banded PE matmuls
     accumulating a neighbor count in PSUM, minus 16*binary, then a
     saturated sigmoid on ScalarE emits exact {0.0, 1.0}.
  5. Stores via gpsimd SWDGE in 1 MiB chunks.
Compute runs at half-channel granularity (16 own rows) with
double-buffered mask/dilation tiles so DVE/PE/ScalarE/DMA pipeline.
"""

import os
import sys

import numpy as np

for _p in ("/opt/trn_rl_repo", "/root/.axon_site/_ro/trn_rl_repo"):
    if os.path.isdir(_p) and _p not in sys.path:
        sys.path.insert(0, _p)

import ml_dtypes

B, C, D, H, W = 2, 4, 64, 256, 256
NCORES = 8
HS = H // NCORES  # 32 own rows per core
HA = HS + 2  # rows incl halo
HPAD = np.float32(-1e30)  # halo pad at global H edges -> mask 0

MHW = 258  # mH row width: 256 data cols + 2 zero pad cols
# half-channel mH: 17 rows (1 pad + 16 data) + slack for dw=+1 AP views
MHLEN = 17 * MHW + 2

_TWO_PHASE = False
_CACHE = {}


def _consts():
    bd = np.arange(128)
    b = bd // D
    d = bd % D
    A = (b[:, None] == b[None, :]) & (np.abs(d[:, None] - d[None, :]) <= 1)
    A = A.astype(ml_dtypes.bfloat16)
    negI = (-16.0 * np.eye(128)).astype(ml_dtypes.bfloat16)
    I128 = np.eye(128, dtype=np.float32)
    return A, negI, I128


def _build():
    import concourse.bass as bass
    import concourse.bacc as bacc
    import concourse.mybir as mybir
    import concourse.tile as tile
    from contextlib import ExitStack

    f32 = mybir.dt.float32
    bf16 = mybir.dt.bfloat16
    Alu = mybir.AluOpType

    nc = bacc.Bacc(
        "TRN2",
        target_bir_lowering=False,
        debug=False,
        num_devices=NCORES,
    )

    xs = nc.dram_tensor("xs", [B, C, D, HA, W], f32, kind="ExternalInput")
    out = nc.dram_tensor("out", [B, C, D, HS, W], f32, kind="ExternalOutput")
    A_np, negI_np, I_np = _consts()
    bandA_d = nc.inline_tensor(A_np, name="bandA")
    negI_d = nc.inline_tensor(negI_np, name="negI")
    ident_d = nc.inline_tensor(I_np, name="ident")

    # partition axis = (b, d) = 128; DRAM-side APs keep b and d as separate
    # leading dims (DMA pairs elements in iteration order, b-major then d,
    # matching partition index p = b*64 + d).
    xsa = xs.ap()
    outa = out.ap()

    with ExitStack() as ctx:
        tc = ctx.enter_context(tile.TileContext(nc))
        pers = ctx.enter_context(tc.tile_pool(name="pers", bufs=1))
        maskp = ctx.enter_context(tc.tile_pool(name="mask", bufs=2))
        stagp = ctx.enter_context(tc.tile_pool(name="stag", bufs=2))
        psump = ctx.enter_context(tc.tile_pool(name="psum", bufs=2, space="PSUM"))
        dramp = ctx.enter_context(tc.tile_pool(name="dram", bufs=1, space="DRAM"))

        x_all = pers.tile([128, C, HA, W], f32)  # 136 KiB / partition
        pmax = pers.tile([128, 8], f32)
        pmin = pers.tile([128, 8], f32)
        red8 = pers.tile([128, 8], f32)  # [mx(4) | -mn(4)] local
        s8 = pers.tile([128, 1], f32)  # per-partition reduced (parts 0..7)
        s1v = pers.tile([128, 8], f32)  # allreduced vals on partition 0
        gv8 = pers.tile([128, 8], f32)  # broadcast [mx | -mn] on all parts
        mnv = pers.tile([128, 4], f32)  # mn per channel
        h4 = pers.tile([128, 4], f32)  # 0.5*(mx-mn) per channel
        At = pers.tile([128, 128], bf16)
        Nt = pers.tile([128, 128], bf16)
        It = pers.tile([128, 128], f32)
        ones1 = pers.tile([128, 128], f32)  # row 0 used as all-ones lhsT
        sel_bias = pers.tile([128, 1], f32)

        cc_in = dramp.tile([8, 1], f32, tag="ccin")
        cc_out = dramp.tile([8, 1], f32, tag="ccout")

        nc.vector.memset(sel_bias[:, :], -100.0)
        nc.vector.memset(ones1[:, :], 1.0)
        nc.sync.dma_start(out=At[:, :], in_=bandA_d.ap())
        nc.sync.dma_start(out=Nt[:, :], in_=negI_d.ap())
        nc.sync.dma_start(out=It[:, :], in_=ident_d.ap())

        # ---- bulk loads: 8 chunks of 2.2 MiB on the gpsimd SWDGE queue ----
        # FIFO ring order staggers completions so per-chunk reduces overlap.
        for c in range(C):
            for hf in range(2):
                nc.gpsimd.dma_start(
                    out=x_all[:, c, 17 * hf : 17 * hf + 17, :],
                    in_=xsa[:, c, :, 17 * hf : 17 * hf + 17, :],
                )

        # ---- per-chunk min/max partials on DVE (own rows only) ----
        for c in range(C):
            for hf in range(2):
                k = 2 * c + hf
                rows = x_all[:, c, 1 + 16 * hf : 17 + 16 * hf, :]
                nc.vector.tensor_reduce(
                    out=pmax[:, k : k + 1],
                    in_=rows,
                    axis=mybir.AxisListType.XY,
                    op=Alu.max,
                )
                nc.vector.tensor_reduce(
                    out=pmin[:, k : k + 1],
                    in_=rows,
                    axis=mybir.AxisListType.XY,
                    op=Alu.min,
                )
        for c in range(C):
            nc.vector.tensor_reduce(
                out=red8[:, c : c + 1],
                in_=pmax[:, 2 * c : 2 * c + 2],
                axis=mybir.AxisListType.X,
                op=Alu.max,
            )
            nc.vector.tensor_reduce(
                out=red8[:, 4 + c : 5 + c],
                in_=pmin[:, 2 * c : 2 * c + 2],
                axis=mybir.AxisListType.X,
                op=Alu.min,
            )
        # negate the mins so a single max combines both downstream
        nc.vector.tensor_scalar_mul(red8[:, 4:8], red8[:, 4:8], -1.0)
        # cross-partition max: transpose red8 [128p, 8] -> psum [8p, 128] with
        # a PE matmul against the identity, then free-axis reduce on DVE.
        pst = psump.tile([128, 2048], f32, tag="ps")
        nc.tensor.matmul(pst[0:8, 0:128], red8[:, :], It[:, :], start=True, stop=True)
        nc.vector.tensor_reduce(
            out=s8[0:8, 0:1],
            in_=pst[0:8, 0:128],
            axis=mybir.AxisListType.X,
            op=Alu.max,
        )

        # ---- 8-core AllReduce(max) over the 8 partials ----
        nc.sync.dma_start(out=cc_in[:, :], in_=s8[0:8, 0:1])
        nc.gpsimd.collective_compute(
            "AllReduce",
            Alu.max,
            replica_groups=[list(range(NCORES))],
            ins=[cc_in[:, :].opt()],
            outs=[cc_out[:, :].opt()],
        )
        nc.sync.dma_start(
            out=s1v[0:1, 0:8],
            in_=cc_out[:, :].rearrange("k j -> (k j)")[None, :],
        )
        # broadcast to all 128 partitions with a rank-1 matmul
        psb = psump.tile([128, 2048], f32, tag="ps")
        nc.tensor.matmul(psb[:, 0:8], ones1[0:1, :], s1v[0:1, 0:8], start=True, stop=True)
        nc.vector.tensor_copy(gv8[:, :], psb[:, 0:8])
        nc.vector.tensor_scalar_mul(mnv[:, :], gv8[:, 4:8], -1.0)
        nc.vector.tensor_add(h4[:, :], gv8[:, 0:4], gv8[:, 4:8])
        nc.vector.tensor_scalar_mul(h4[:, :], h4[:, :], 0.5)

        # ---- mask, dilate, boundary: half-channel (16 own rows) pipeline ----
        for c in range(C):
            for hf in range(2):
                # binm rows 0..17 = x halo rows 16*hf .. 16*hf+17
                binm = maskp.tile([128, 18, W], bf16, tag="bin")
                mH = maskp.tile([128, MHLEN], bf16, tag="mh")
                if c == 0:
                    # zero the pad columns once per buffer (bufs=2 -> c=0
                    # touches both buffers; later iters only rewrite data)
                    nc.vector.memset(mH[:, :], 0.0)
                nc.vector.tensor_scalar(
                    out=binm[:, :, :],
                    in0=x_all[:, c, 16 * hf : 16 * hf + 18, :],
                    scalar1=mnv[:, c : c + 1],
                    scalar2=h4[:, c : c + 1],
                    op0=Alu.subtract,
                    op1=Alu.is_gt,
                )
                mHd = mH[:, MHW : MHW + 16 * MHW].rearrange(
                    "p (r z) -> p r z", z=MHW
                )[:, :, 0:W]
                nc.vector.tensor_tensor(
                    out=mHd,
                    in0=binm[:, 0:16, :],
                    in1=binm[:, 2:18, :],
                    op=Alu.max,
                )
                nc.vector.tensor_tensor(
                    out=mHd,
                    in0=mHd,
                    in1=binm[:, 1:17, :],
                    op=Alu.max,
                )
                for t in range(2):  # 8 own rows per PSUM tile / store
                    ps = psump.tile([128, 2048], f32, tag="ps")
                    for s in range(4):  # one PSUM bank = 2 rows = 512
                        R = 8 * t + 2 * s  # own-row index within the half
                        pslice = ps[:, 512 * s : 512 * s + 512]
                        for j, dw in enumerate((-1, 0, 1)):
                            off = (R + 1) * MHW + dw
                            rhs = mH[:, off : off + 2 * MHW].rearrange(
                                "p (r z) -> p r z", z=MHW
                            )[:, :, 0:W]
                            nc.tensor.matmul(
                                pslice,
                                At[:, :],
                                rhs,
                                start=(j == 0),
                                stop=False,
                            )
                        nc.tensor.matmul(
                            pslice,
                            Nt[:, :],
                            binm[:, 1 + R : 3 + R, :],
                            start=False,
                            stop=True,
                        )
                    stg = stagp.tile([128, 2048], f32, tag="st")
                    nc.scalar.activation(
                        out=stg[:, :],
                        in_=ps[:, :],
                        func=mybir.ActivationFunctionType.Sigmoid,
                        bias=sel_bias[:, :],
                        scale=200.0,
                    )
                    r0 = 16 * hf + 8 * t  # own-row base in the core's shard
                    nc.gpsimd.dma_start(
                        out=outa[:, c, :, r0 : r0 + 8, :],
                        in_=stg.rearrange("p (r w) -> p r w", w=W),
                    )

    nc.compile()
    return nc


def _get_nc():
    if "nc" not in _CACHE:
        _CACHE["nc"] = _build()
    return _CACHE["nc"]


def _make_in_maps(x: np.ndarray):
    in_maps = []
    for k in range(NCORES):
        xs = np.empty((B, C, D, HA, W), np.float32)
        lo = k * HS
        xs[:, :, :, 1 : HS + 1, :] = x[:, :, :, lo : lo + HS, :]
        if k > 0:
            xs[:, :, :, 0, :] = x[:, :, :, lo - 1, :]
        else:
            xs[:, :, :, 0, :] = HPAD
        if k < NCORES - 1:
            xs[:, :, :, HS + 1, :] = x[:, :, :, lo + HS, :]
        else:
            xs[:, :, :, HS + 1, :] = HPAD
        in_maps.append({"xs": xs})
    return in_maps


def kernel(x: np.ndarray) -> np.ndarray:
    from concourse.bass_utils import run_bass_kernel_spmd

    x = np.ascontiguousarray(np.asarray(x), dtype=np.float32)
    assert x.shape == (B, C, D, H, W)

    in_maps = _make_in_maps(x)
    res = run_bass_kernel_spmd(_get_nc(), in_maps, core_ids=list(range(NCORES)))
    pieces = [res.results[k]["out"] for k in range(NCORES)]
    return np.concatenate(pieces, axis=3)


if __name__ == "__main__":
    x = np.random.randn(B, C, D, H, W).astype(np.float32)
    y = kernel(x)
    print(y.shape, y.dtype, y.sum())


# revision 7
# speedup vs baseline: 4.6165x; 2.4561x over previous
"""Boundary rendering module for Trainium2 (8 NeuronCores), fused single launch.

Computes, for x of shape (2, 4, 64, 256, 256) f32:
    mn/mx  = per-channel global min/max
    binary = ((x - mn) / (mx - mn)) > 0.5     [== (x - mn) > 0.5*(mx - mn)]
    dilated = 3x3x3 binary dilation of binary (SAME padding)
    out    = dilated - binary

Sharding: H (=256) split into 8 chunks of 32 rows, one per NeuronCore.
Each core receives its 32 rows plus one halo row on each side (global
edges padded with -1e30 so the halo mask is 0).  On-core layout puts
(B, D) = 128 on the SBUF partition axis; (C, H, W) live on the free axis.

Single launch per core:
  1. x loaded in 8 (channel, half) chunks via gpsimd SWDGE DMA (spreads
     across all 16 SDMA engines; the 2 HWDGE rings top out at ~75 GB/s).
  2. DVE min/max partials per chunk as each load lands (overlapped).
  3. Partials transposed across partitions with a PE identity matmul,
     reduced, then an 8-core AllReduce(max) over [mx(4) | -mn(4)]
     through DRAM bounce buffers; result broadcast back to all 128
     partitions with a rank-1 PE matmul.
  4. Mask + H-dilation on DVE (bf16), W+D dilation as banded PE matmuls
     accumulating a neighbor count in PSUM, minus 16*binary, then a
     saturated sigmoid on ScalarE emits exact {0.0, 1.0}.
  5. Stores via gpsimd SWDGE in 1 MiB chunks.
Compute runs at half-channel granularity (16 own rows) with
double-buffered mask/dilation tiles so DVE/PE/ScalarE/DMA pipeline.
"""

import os
import sys

import numpy as np

for _p in ("/opt/trn_rl_repo", "/root/.axon_site/_ro/trn_rl_repo"):
    if os.path.isdir(_p) and _p not in sys.path:
        sys.path.insert(0, _p)

import ml_dtypes

B, C, D, H, W = 2, 4, 64, 256, 256
NCORES = 8
HS = H // NCORES  # 32 own rows per core
HA = HS + 2  # rows incl halo
HPAD = np.float32(-1e30)  # halo pad at global H edges -> mask 0

MHW = 258  # mH row width: 256 data cols + 2 zero pad cols
# half-channel mH: 17 rows (1 pad + 16 data) + slack for dw=+1 AP views
MHLEN = 17 * MHW + 2

# flat per-partition sizes: DRAM shards use layout [B, D, C, H', W] so each
# partition (b, d) owns one contiguous run -> 2-dim DMA APs.  Strided 3-dim
# APs run ~4x slower through SWDGE (measured 244us vs 56us for the 17.8 MiB
# load) and defeat chunk-completion staggering.
XPP = C * HA * W  # 34816 f32 per partition (input incl halo)
OPP = C * HS * W  # 32768 f32 per partition (output)

_TWO_PHASE = False
_CACHE = {}


def _consts():
    bd = np.arange(128)
    b = bd // D
    d = bd % D
    A = (b[:, None] == b[None, :]) & (np.abs(d[:, None] - d[None, :]) <= 1)
    A = A.astype(ml_dtypes.bfloat16)
    negI = (-16.0 * np.eye(128)).astype(ml_dtypes.bfloat16)
    I128 = np.eye(128, dtype=np.float32)
    return A, negI, I128


def _build():
    import concourse.bass as bass
    import concourse.bacc as bacc
    import concourse.mybir as mybir
    import concourse.tile as tile
    from contextlib import ExitStack

    f32 = mybir.dt.float32
    bf16 = mybir.dt.bfloat16
    Alu = mybir.AluOpType

    nc = bacc.Bacc(
        "TRN2",
        target_bir_lowering=False,
        debug=False,
        num_devices=NCORES,
    )

    xs = nc.dram_tensor("xs", [B, D, XPP], f32, kind="ExternalInput")
    out = nc.dram_tensor("out", [B, D, OPP], f32, kind="ExternalOutput")
    A_np, negI_np, I_np = _consts()
    bandA_d = nc.inline_tensor(A_np, name="bandA")
    negI_d = nc.inline_tensor(negI_np, name="negI")
    ident_d = nc.inline_tensor(I_np, name="ident")

    # partition axis = (b, d) = 128; the (b, d) DRAM dims merge into a single
    # 128-partition dim during AP optimization since b's stride = 64 * d's.
    xsa = xs.ap()
    outa = out.ap()

    with ExitStack() as ctx:
        tc = ctx.enter_context(tile.TileContext(nc))
        pers = ctx.enter_context(tc.tile_pool(name="pers", bufs=1))
        maskp = ctx.enter_context(tc.tile_pool(name="mask", bufs=2))
        stagp = ctx.enter_context(tc.tile_pool(name="stag", bufs=2))
        psump = ctx.enter_context(tc.tile_pool(name="psum", bufs=2, space="PSUM"))
        dramp = ctx.enter_context(tc.tile_pool(name="dram", bufs=1, space="DRAM"))

        x_all = pers.tile([128, C, HA, W], f32)  # 136 KiB / partition
        pmax = pers.tile([128, 8], f32)
        pmin = pers.tile([128, 8], f32)
        red8 = pers.tile([128, 8], f32)  # [mx(4) | -mn(4)] local
        s8 = pers.tile([128, 1], f32)  # per-partition reduced (parts 0..7)
        s1v = pers.tile([128, 8], f32)  # allreduced vals on partition 0
        gv8 = pers.tile([128, 8], f32)  # broadcast [mx | -mn] on all parts
        mnv = pers.tile([128, 4], f32)  # mn per channel
        h4 = pers.tile([128, 4], f32)  # 0.5*(mx-mn) per channel
        At = pers.tile([128, 128], bf16)
        Nt = pers.tile([128, 128], bf16)
        It = pers.tile([128, 128], f32)
        ones1 = pers.tile([128, 128], f32)  # row 0 used as all-ones lhsT
        sel_bias = pers.tile([128, 1], f32)

        cc_in = dramp.tile([8, 1], f32, tag="ccin")
        cc_out = dramp.tile([8, 1], f32, tag="ccout")

        nc.vector.memset(sel_bias[:, :], -100.0)
        nc.vector.memset(ones1[:, :], 1.0)
        nc.sync.dma_start(out=At[:, :], in_=bandA_d.ap())
        nc.sync.dma_start(out=Nt[:, :], in_=negI_d.ap())
        nc.sync.dma_start(out=It[:, :], in_=ident_d.ap())

        # ---- bulk loads: 8 chunks of 2.2 MiB on the gpsimd SWDGE queue ----
        # FIFO ring order staggers completions so per-chunk reduces overlap.
        xv = x_all[:].rearrange("p c h w -> p (c h w)")
        for c in range(C):
            for hf in range(2):
                o = c * (HA * W) + hf * 17 * W
                nc.gpsimd.dma_start(
                    out=xv[:, o : o + 17 * W],
                    in_=xsa[:, :, o : o + 17 * W],
                )

        # ---- per-chunk min/max partials on DVE (own rows only) ----
        for c in range(C):
            for hf in range(2):
                k = 2 * c + hf
                rows = x_all[:, c, 1 + 16 * hf : 17 + 16 * hf, :]
                nc.vector.tensor_reduce(
                    out=pmax[:, k : k + 1],
                    in_=rows,
                    axis=mybir.AxisListType.XY,
                    op=Alu.max,
                )
                nc.vector.tensor_reduce(
                    out=pmin[:, k : k + 1],
                    in_=rows,
                    axis=mybir.AxisListType.XY,
                    op=Alu.min,
                )
        for c in range(C):
            nc.vector.tensor_reduce(
                out=red8[:, c : c + 1],
                in_=pmax[:, 2 * c : 2 * c + 2],
                axis=mybir.AxisListType.X,
                op=Alu.max,
            )
            nc.vector.tensor_reduce(
                out=red8[:, 4 + c : 5 + c],
                in_=pmin[:, 2 * c : 2 * c + 2],
                axis=mybir.AxisListType.X,
                op=Alu.min,
            )
        # negate the mins so a single max combines both downstream
        nc.vector.tensor_scalar_mul(red8[:, 4:8], red8[:, 4:8], -1.0)
        # cross-partition max: transpose red8 [128p, 8] -> psum [8p, 128] with
        # a PE matmul against the identity, then free-axis reduce on DVE.
        pst = psump.tile([128, 2048], f32, tag="ps")
        nc.tensor.matmul(pst[0:8, 0:128], red8[:, :], It[:, :], start=True, stop=True)
        nc.vector.tensor_reduce(
            out=s8[0:8, 0:1],
            in_=pst[0:8, 0:128],
            axis=mybir.AxisListType.X,
            op=Alu.max,
        )

        # ---- 8-core AllReduce(max) over the 8 partials ----
        nc.sync.dma_start(out=cc_in[:, :], in_=s8[0:8, 0:1])
        nc.gpsimd.collective_compute(
            "AllReduce",
            Alu.max,
            replica_groups=[list(range(NCORES))],
            ins=[cc_in[:, :].opt()],
            outs=[cc_out[:, :].opt()],
        )
        nc.sync.dma_start(
            out=s1v[0:1, 0:8],
            in_=cc_out[:, :].rearrange("k j -> (k j)")[None, :],
        )
        # broadcast to all 128 partitions with a rank-1 matmul
        psb = psump.tile([128, 2048], f32, tag="ps")
        nc.tensor.matmul(psb[:, 0:8], ones1[0:1, :], s1v[0:1, 0:8], start=True, stop=True)
        nc.vector.tensor_copy(gv8[:, :], psb[:, 0:8])
        nc.vector.tensor_scalar_mul(mnv[:, :], gv8[:, 4:8], -1.0)
        nc.vector.tensor_add(h4[:, :], gv8[:, 0:4], gv8[:, 4:8])
        nc.vector.tensor_scalar_mul(h4[:, :], h4[:, :], 0.5)

        # ---- mask, dilate, boundary: half-channel (16 own rows) pipeline ----
        for c in range(C):
            for hf in range(2):
                # binm rows 0..17 = x halo rows 16*hf .. 16*hf+17
                binm = maskp.tile([128, 18, W], bf16, tag="bin")
                mH = maskp.tile([128, MHLEN], bf16, tag="mh")
                if c == 0:
                    # zero the pad columns once per buffer (bufs=2 -> c=0
                    # touches both buffers; later iters only rewrite data)
                    nc.vector.memset(mH[:, :], 0.0)
                nc.vector.tensor_scalar(
                    out=binm[:, :, :],
                    in0=x_all[:, c, 16 * hf : 16 * hf + 18, :],
                    scalar1=mnv[:, c : c + 1],
                    scalar2=h4[:, c : c + 1],
                    op0=Alu.subtract,
                    op1=Alu.is_gt,
                )
                mHd = mH[:, MHW : MHW + 16 * MHW].rearrange(
                    "p (r z) -> p r z", z=MHW
                )[:, :, 0:W]
                nc.vector.tensor_tensor(
                    out=mHd,
                    in0=binm[:, 0:16, :],
                    in1=binm[:, 2:18, :],
                    op=Alu.max,
                )
                nc.vector.tensor_tensor(
                    out=mHd,
                    in0=mHd,
                    in1=binm[:, 1:17, :],
                    op=Alu.max,
                )
                for t in range(2):  # 8 own rows per PSUM tile / store
                    ps = psump.tile([128, 2048], f32, tag="ps")
                    for s in range(4):  # one PSUM bank = 2 rows = 512
                        R = 8 * t + 2 * s  # own-row index within the half
                        pslice = ps[:, 512 * s : 512 * s + 512]
                        for j, dw in enumerate((-1, 0, 1)):
                            off = (R + 1) * MHW + dw
                            rhs = mH[:, off : off + 2 * MHW].rearrange(
                                "p (r z) -> p r z", z=MHW
                            )[:, :, 0:W]
                            nc.tensor.matmul(
                                pslice,
                                At[:, :],
                                rhs,
                                start=(j == 0),
                                stop=False,
                            )
                        nc.tensor.matmul(
                            pslice,
                            Nt[:, :],
                            binm[:, 1 + R : 3 + R, :],
                            start=False,
                            stop=True,
                        )
                    stg = stagp.tile([128, 2048], f32, tag="st")
                    nc.scalar.activation(
                        out=stg[:, :],
                        in_=ps[:, :],
                        func=mybir.ActivationFunctionType.Sigmoid,
                        bias=sel_bias[:, :],
                        scale=200.0,
                    )
                    r0 = 16 * hf + 8 * t  # own-row base in the core's shard
                    oo = c * (HS * W) + r0 * W
                    nc.gpsimd.dma_start(
                        out=outa[:, :, oo : oo + 8 * W],
                        in_=stg[:, :],
                    )

    nc.compile()
    return nc


def _get_nc():
    if "nc" not in _CACHE:
        _CACHE["nc"] = _build()
    return _CACHE["nc"]


def _make_in_maps(x: np.ndarray):
    # device shards use layout [B, D, C, HA, W] (flat per partition)
    xt = np.ascontiguousarray(x.transpose(0, 2, 1, 3, 4))  # [B, D, C, H, W]
    in_maps = []
    for k in range(NCORES):
        xs = np.empty((B, D, C, HA, W), np.float32)
        lo = k * HS
        xs[:, :, :, 1 : HS + 1, :] = xt[:, :, :, lo : lo + HS, :]
        if k > 0:
            xs[:, :, :, 0, :] = xt[:, :, :, lo - 1, :]
        else:
            xs[:, :, :, 0, :] = HPAD
        if k < NCORES - 1:
            xs[:, :, :, HS + 1, :] = xt[:, :, :, lo + HS, :]
        else:
            xs[:, :, :, HS + 1, :] = HPAD
        in_maps.append({"xs": xs.reshape(B, D, XPP)})
    return in_maps


def kernel(x: np.ndarray) -> np.ndarray:
    from concourse.bass_utils import run_bass_kernel_spmd

    x = np.ascontiguousarray(np.asarray(x), dtype=np.float32)
    assert x.shape == (B, C, D, H, W)

    in_maps = _make_in_maps(x)
    res = run_bass_kernel_spmd(_get_nc(), in_maps, core_ids=list(range(NCORES)))
    # shard outs are [B, D, C, HS, W]; back to [B, C, D, HS, W], concat on H
    pieces = [
        res.results[k]["out"].reshape(B, D, C, HS, W).transpose(0, 2, 1, 3, 4)
        for k in range(NCORES)
    ]
    return np.concatenate(pieces, axis=3)


if __name__ == "__main__":
    x = np.random.randn(B, C, D, H, W).astype(np.float32)
    y = kernel(x)
    print(y.shape, y.dtype, y.sum())


# revision 12
# speedup vs baseline: 4.9993x; 1.0829x over previous
"""Boundary rendering module for Trainium2 (8 NeuronCores), fused single launch.

Computes, for x of shape (2, 4, 64, 256, 256) f32:
    mn/mx  = per-channel global min/max
    binary = ((x - mn) / (mx - mn)) > 0.5     [== (x - mn) > 0.5*(mx - mn)]
    dilated = 3x3x3 binary dilation of binary (SAME padding)
    out    = dilated - binary

Sharding: H (=256) split into 8 chunks of 32 rows, one per NeuronCore.
Each core receives its 32 rows plus one halo row on each side (global
edges padded with -1e30 so the halo mask is 0).  On-core layout puts
(B, D) = 128 on the SBUF partition axis; (C, H, W) live on the free axis.

Single launch per core:
  1. x loaded in 8 (channel, half) chunks via gpsimd SWDGE DMA (spreads
     across all 16 SDMA engines; the 2 HWDGE rings top out at ~75 GB/s).
  2. DVE min/max partials per chunk as each load lands (overlapped).
  3. Partials transposed across partitions with a PE identity matmul,
     reduced, then an 8-core AllReduce(max) over [mx(4) | -mn(4)]
     through DRAM bounce buffers; result broadcast back to all 128
     partitions with a rank-1 PE matmul.
  4. Mask + H-dilation on DVE (bf16), W+D dilation as banded PE matmuls
     accumulating a neighbor count in PSUM, minus 16*binary, then a
     saturated sigmoid on ScalarE emits exact {0.0, 1.0}.
  5. Stores via gpsimd SWDGE in 1 MiB chunks.
Compute runs at half-channel granularity (16 own rows) with
double-buffered mask/dilation tiles so DVE/PE/ScalarE/DMA pipeline.
"""

import os
import sys

import numpy as np

for _p in ("/opt/trn_rl_repo", "/root/.axon_site/_ro/trn_rl_repo"):
    if os.path.isdir(_p) and _p not in sys.path:
        sys.path.insert(0, _p)

import ml_dtypes

B, C, D, H, W = 2, 4, 64, 256, 256
NCORES = 8
HS = H // NCORES  # 32 own rows per core
HA = HS + 2  # rows incl halo
HPAD = np.float32(-1e30)  # halo pad at global H edges -> mask 0

MHW = 258  # mH row width: 256 data cols + 2 zero pad cols
# half-channel mH: 17 rows (1 pad + 16 data) + slack for dw=+1 AP views
MHLEN = 17 * MHW + 2

# flat per-partition sizes: DRAM shards use layout [B, D, C, H', W] so each
# partition (b, d) owns one contiguous run -> 2-dim DMA APs.  Strided 3-dim
# APs run ~4x slower through SWDGE (measured 244us vs 56us for the 17.8 MiB
# load) and defeat chunk-completion staggering.
XPP = C * HA * W  # 34816 f32 per partition (input incl halo)
OPP = C * HS * W  # 32768 f32 per partition (output)

_TWO_PHASE = False
_CACHE = {}


def _consts():
    bd = np.arange(128)
    b = bd // D
    d = bd % D
    A = (b[:, None] == b[None, :]) & (np.abs(d[:, None] - d[None, :]) <= 1)
    A = A.astype(ml_dtypes.bfloat16)
    negI = (-16.0 * np.eye(128)).astype(ml_dtypes.bfloat16)
    I128 = np.eye(128, dtype=np.float32)
    return A, negI, I128


def _build():
    import concourse.bass as bass
    import concourse.bacc as bacc
    import concourse.mybir as mybir
    import concourse.tile as tile
    from contextlib import ExitStack

    f32 = mybir.dt.float32
    bf16 = mybir.dt.bfloat16
    Alu = mybir.AluOpType

    nc = bacc.Bacc(
        "TRN2",
        target_bir_lowering=False,
        debug=False,
        num_devices=NCORES,
    )

    xs = nc.dram_tensor("xs", [B, D, XPP], f32, kind="ExternalInput")
    out = nc.dram_tensor("out", [B, D, OPP], f32, kind="ExternalOutput")
    A_np, negI_np, I_np = _consts()
    bandA_d = nc.inline_tensor(A_np, name="bandA")
    negI_d = nc.inline_tensor(negI_np, name="negI")
    ident_d = nc.inline_tensor(I_np, name="ident")

    # partition axis = (b, d) = 128; the (b, d) DRAM dims merge into a single
    # 128-partition dim during AP optimization since b's stride = 64 * d's.
    xsa = xs.ap()
    outa = out.ap()

    with ExitStack() as ctx:
        tc = ctx.enter_context(tile.TileContext(nc))
        pers = ctx.enter_context(tc.tile_pool(name="pers", bufs=1))
        maskp = ctx.enter_context(tc.tile_pool(name="mask", bufs=2))
        stagp = ctx.enter_context(tc.tile_pool(name="stag", bufs=2))
        psump = ctx.enter_context(tc.tile_pool(name="psum", bufs=3, space="PSUM"))
        psmall = ctx.enter_context(tc.tile_pool(name="psm", bufs=2, space="PSUM"))
        dramp = ctx.enter_context(tc.tile_pool(name="dram", bufs=1, space="DRAM"))

        x_all = pers.tile([128, C, HA, W], f32)  # 136 KiB / partition
        pmax = pers.tile([128, 8], f32)
        pmin = pers.tile([128, 8], f32)
        red8 = pers.tile([128, 8], f32)  # per channel c: [mx_c, -mn_c] at 2c
        s4 = pers.tile([128, 4], f32)  # per-partition reduced (parts 0..3)
        s1v = pers.tile([128, 8], f32)  # allreduced vals on partition 0
        gv4 = pers.tile([128, 4], f32)  # broadcast [mx, -mn] x 2 channels
        mnv = pers.tile([128, 4], f32)  # mn per channel
        h4 = pers.tile([128, 4], f32)  # 0.5*(mx-mn) per channel
        At = pers.tile([128, 128], bf16)
        Nt = pers.tile([128, 128], bf16)
        It = pers.tile([128, 128], f32)
        ones1 = pers.tile([128, 128], f32)  # row 0 used as all-ones lhsT
        sel_bias = pers.tile([128, 1], f32)

        nc.vector.memset(sel_bias[:, :], -100.0)
        nc.vector.memset(ones1[:, :], 1.0)
        nc.sync.dma_start(out=At[:, :], in_=bandA_d.ap())
        nc.sync.dma_start(out=Nt[:, :], in_=negI_d.ap())
        nc.sync.dma_start(out=It[:, :], in_=ident_d.ap())

        # ---- bulk loads: 8 chunks of 2.2 MiB on the gpsimd SWDGE queue ----
        # FIFO ring order staggers completions so per-chunk reduces overlap.
        xv = x_all[:].rearrange("p c h w -> p (c h w)")
        for c in range(C):
            for hf in range(2):
                o = c * (HA * W) + hf * 17 * W
                nc.gpsimd.dma_start(
                    out=xv[:, o : o + 17 * W],
                    in_=xsa[:, :, o : o + 17 * W],
                )

        # ---- phase 1: min/max partials + 2 pipelined AllReduces ----
        # Per chunk: DVE takes the max pass; gpsimd (otherwise idle) folds the
        # min pass down to 1024 with two tensor_tensor mins, DVE finishes it.
        # Channels are grouped (0,1) and (2,3); each group gets its own
        # AllReduce so group-a thresholds land ~30us before group-b's and the
        # mask/dilate pipeline starts early.
        groups = [(0, 1), (2, 3)]
        for g, chans in enumerate(groups):
            for c in chans:
                for hf in range(2):
                    k = 2 * c + hf
                    # own rows of the chunk as a flat contiguous view
                    o1 = c * (HA * W) + (1 + 16 * hf) * W
                    rows = xv[:, o1 : o1 + 16 * W]
                    nc.vector.tensor_reduce(
                        out=pmax[:, k : k + 1],
                        in_=rows,
                        axis=mybir.AxisListType.X,
                        op=Alu.max,
                    )
                    nc.vector.tensor_reduce(
                        out=pmin[:, k : k + 1],
                        in_=rows,
                        axis=mybir.AxisListType.X,
                        op=Alu.min,
                    )
                nc.vector.tensor_reduce(
                    out=red8[:, 2 * c : 2 * c + 1],
                    in_=pmax[:, 2 * c : 2 * c + 2],
                    axis=mybir.AxisListType.X,
                    op=Alu.max,
                )
                nc.vector.tensor_reduce(
                    out=red8[:, 2 * c + 1 : 2 * c + 2],
                    in_=pmin[:, 2 * c : 2 * c + 2],
                    axis=mybir.AxisListType.X,
                    op=Alu.min,
                )
                # negate the min so a single AllReduce(max) covers both
                nc.vector.tensor_scalar_mul(
                    red8[:, 2 * c + 1 : 2 * c + 2],
                    red8[:, 2 * c + 1 : 2 * c + 2],
                    -1.0,
                )
            c0 = chans[0]
            # transpose [128p, 4] -> psum [4p, 128] via identity matmul, then
            # free-axis max on DVE -> s4[4g?..], DMA out, AllReduce(max).
            pst = psmall.tile([128, 512], f32, tag="pss")
            nc.tensor.matmul(
                pst[0:4, 0:128],
                red8[:, 2 * c0 : 2 * c0 + 4],
                It[:, :],
                start=True,
                stop=True,
            )
            nc.vector.tensor_reduce(
                out=s4[0:4, g : g + 1],
                in_=pst[0:4, 0:128],
                axis=mybir.AxisListType.X,
                op=Alu.max,
            )
            cc_in = dramp.tile([4, 1], f32, tag=f"ccin{g}")
            cc_out = dramp.tile([4, 1], f32, tag=f"ccout{g}")
            nc.scalar.dma_start(out=cc_in[:, :], in_=s4[0:4, g : g + 1])
            nc.gpsimd.collective_compute(
                "AllReduce",
                Alu.max,
                replica_groups=[list(range(NCORES))],
                ins=[cc_in[:, :].opt()],
                outs=[cc_out[:, :].opt()],
            )
            nc.sync.dma_start(
                out=s1v[0:1, 4 * g : 4 * g + 4],
                in_=cc_out[:, :].rearrange("k j -> (k j)")[None, :],
            )

        # ---- phase 2: per group: thresholds, mask, dilate, boundary ----
        for g, chans in enumerate(groups):
            # broadcast [mx0, -mn0, mx1, -mn1] to all 128 partitions
            psb = psmall.tile([128, 512], f32, tag="pss")
            nc.tensor.matmul(
                psb[:, 0:4],
                ones1[0:1, :],
                s1v[0:1, 4 * g : 4 * g + 4],
                start=True,
                stop=True,
            )
            nc.vector.tensor_copy(gv4[:, :], psb[:, 0:4])
            for j, c in enumerate(chans):
                nc.vector.tensor_scalar_mul(
                    mnv[:, c : c + 1], gv4[:, 2 * j + 1 : 2 * j + 2], -1.0
                )
                nc.vector.tensor_add(
                    h4[:, c : c + 1],
                    gv4[:, 2 * j : 2 * j + 1],
                    gv4[:, 2 * j + 1 : 2 * j + 2],
                )
                nc.vector.tensor_scalar_mul(h4[:, c : c + 1], h4[:, c : c + 1], 0.5)
            for c in chans:
                for hf in range(2):
                    # binm rows 0..17 = x halo rows 16*hf .. 16*hf+17
                    binm = maskp.tile([128, 18, W], bf16, tag="bin")
                    mH = maskp.tile([128, MHLEN], bf16, tag="mh")
                    if c == chans[0] and g == 0:
                        # zero the pad columns once per buffer (bufs=2 -> the
                        # first two iterations touch both buffers)
                        nc.vector.memset(mH[:, :], 0.0)
                    nc.vector.tensor_scalar(
                        out=binm[:, :, :],
                        in0=x_all[:, c, 16 * hf : 16 * hf + 18, :],
                        scalar1=mnv[:, c : c + 1],
                        scalar2=h4[:, c : c + 1],
                        op0=Alu.subtract,
                        op1=Alu.is_gt,
                    )
                    mHd = mH[:, MHW : MHW + 16 * MHW].rearrange(
                        "p (r z) -> p r z", z=MHW
                    )[:, :, 0:W]
                    nc.vector.tensor_tensor(
                        out=mHd,
                        in0=binm[:, 0:16, :],
                        in1=binm[:, 2:18, :],
                        op=Alu.max,
                    )
                    nc.vector.tensor_tensor(
                        out=mHd,
                        in0=mHd,
                        in1=binm[:, 1:17, :],
                        op=Alu.max,
                    )
                    for tp in range(2):  # 8 own rows per staging buffer
                        stg = stagp.tile([128, 2048], f32, tag="st")
                        for t in range(2):  # 4 own rows per PSUM tile
                            ps = psump.tile([128, 1024], f32, tag="ps")
                            for s in range(2):  # one PSUM bank = 2 rows
                                R = 8 * tp + 4 * t + 2 * s
                                pslice = ps[:, 512 * s : 512 * s + 512]
                                for jj, dw in enumerate((-1, 0, 1)):
                                    off = (R + 1) * MHW + dw
                                    rhs = mH[:, off : off + 2 * MHW].rearrange(
                                        "p (r z) -> p r z", z=MHW
                                    )[:, :, 0:W]
                                    nc.tensor.matmul(
                                        pslice,
                                        At[:, :],
                                        rhs,
                                        start=(jj == 0),
                                        stop=False,
                                    )
                                nc.tensor.matmul(
                                    pslice,
                                    Nt[:, :],
                                    binm[:, 1 + R : 3 + R, :],
                                    start=False,
                                    stop=True,
                                )
                            nc.scalar.activation(
                                out=stg[:, 1024 * t : 1024 * t + 1024],
                                in_=ps[:, :],
                                func=mybir.ActivationFunctionType.Sigmoid,
                                bias=sel_bias[:, :],
                                scale=200.0,
                            )
                        r0 = 16 * hf + 8 * tp  # own-row base in the shard
                        oo = c * (HS * W) + r0 * W
                        nc.gpsimd.dma_start(
                            out=outa[:, :, oo : oo + 8 * W],
                            in_=stg[:, :],
                        )

    nc.compile()
    return nc


def _get_nc():
    if "nc" not in _CACHE:
        _CACHE["nc"] = _build()
    return _CACHE["nc"]


def _make_in_maps(x: np.ndarray):
    # device shards use layout [B, D, C, HA, W] (flat per partition)
    xt = np.ascontiguousarray(x.transpose(0, 2, 1, 3, 4))  # [B, D, C, H, W]
    in_maps = []
    for k in range(NCORES):
        xs = np.empty((B, D, C, HA, W), np.float32)
        lo = k * HS
        xs[:, :, :, 1 : HS + 1, :] = xt[:, :, :, lo : lo + HS, :]
        if k > 0:
            xs[:, :, :, 0, :] = xt[:, :, :, lo - 1, :]
        else:
            xs[:, :, :, 0, :] = HPAD
        if k < NCORES - 1:
            xs[:, :, :, HS + 1, :] = xt[:, :, :, lo + HS, :]
        else:
            xs[:, :, :, HS + 1, :] = HPAD
        in_maps.append({"xs": xs.reshape(B, D, XPP)})
    return in_maps


def kernel(x: np.ndarray) -> np.ndarray:
    from concourse.bass_utils import run_bass_kernel_spmd

    x = np.ascontiguousarray(np.asarray(x), dtype=np.float32)
    assert x.shape == (B, C, D, H, W)

    in_maps = _make_in_maps(x)
    res = run_bass_kernel_spmd(_get_nc(), in_maps, core_ids=list(range(NCORES)))
    # shard outs are [B, D, C, HS, W]; back to [B, C, D, HS, W], concat on H
    pieces = [
        res.results[k]["out"].reshape(B, D, C, HS, W).transpose(0, 2, 1, 3, 4)
        for k in range(NCORES)
    ]
    return np.concatenate(pieces, axis=3)


if __name__ == "__main__":
    x = np.random.randn(B, C, D, H, W).astype(np.float32)
    y = kernel(x)
    print(y.shape, y.dtype, y.sum())


# revision 21
# speedup vs baseline: 5.7370x; 1.1476x over previous
"""Boundary rendering module for Trainium2 (8 NeuronCores), fused single launch.

Computes, for x of shape (2, 4, 64, 256, 256) f32:
    mn/mx  = per-channel global min/max
    binary = ((x - mn) / (mx - mn)) > 0.5     [== (x - mn) > 0.5*(mx - mn)]
    dilated = 3x3x3 binary dilation of binary (SAME padding)
    out    = dilated - binary

Sharding: H (=256) split into 8 chunks of 32 rows, one per NeuronCore.
Each core receives its 32 rows plus one halo row on each side (global
edges padded with -1e30 so the halo mask is 0).  On-core layout puts
(B, D) = 128 on the SBUF partition axis; (C, H, W) live on the free axis.

Single launch per core:
  1. x loaded in 8 (channel, half) chunks via gpsimd SWDGE DMA (spreads
     across all 16 SDMA engines; the 2 HWDGE rings top out at ~75 GB/s).
  2. DVE min/max partials per chunk as each load lands (overlapped).
  3. Partials transposed across partitions with a PE identity matmul,
     reduced, then an 8-core AllReduce(max) over [mx(4) | -mn(4)]
     through DRAM bounce buffers; result broadcast back to all 128
     partitions with a rank-1 PE matmul.
  4. Mask + H-dilation on DVE (bf16), W+D dilation as banded PE matmuls
     accumulating a neighbor count in PSUM, minus 16*binary, then a
     saturated sigmoid on ScalarE emits exact {0.0, 1.0}.
  5. Stores via gpsimd SWDGE in 1 MiB chunks.
Compute runs at half-channel granularity (16 own rows) with
double-buffered mask/dilation tiles so DVE/PE/ScalarE/DMA pipeline.
"""

import os
import sys

import numpy as np

for _p in ("/opt/trn_rl_repo", "/root/.axon_site/_ro/trn_rl_repo"):
    if os.path.isdir(_p) and _p not in sys.path:
        sys.path.insert(0, _p)

import ml_dtypes

B, C, D, H, W = 2, 4, 64, 256, 256
NCORES = 8
HS = H // NCORES  # 32 own rows per core
HA = HS + 2  # rows incl halo
HPAD = np.float32(-1e30)  # halo pad at global H edges -> mask 0

MHW = 258  # mH row width: 256 data cols + 2 zero pad cols
# half-channel mH: 17 rows (1 pad + 16 data) + slack for dw=+1 AP views
MHLEN = 17 * MHW + 2

# flat per-partition sizes: DRAM shards use layout [B, D, C, H', W] so each
# partition (b, d) owns one contiguous run -> 2-dim DMA APs.  Strided 3-dim
# APs run ~4x slower through SWDGE (measured 244us vs 56us for the 17.8 MiB
# load) and defeat chunk-completion staggering.
XPP = C * HA * W  # 34816 f32 per partition (input incl halo)
OPP = C * HS * W  # 32768 f32 per partition (output)

_TWO_PHASE = False
_CACHE = {}


def _consts():
    bd = np.arange(128)
    b = bd // D
    d = bd % D
    A = (b[:, None] == b[None, :]) & (np.abs(d[:, None] - d[None, :]) <= 1)
    A = A.astype(ml_dtypes.bfloat16)
    negI = (-16.0 * np.eye(128)).astype(ml_dtypes.bfloat16)
    I128 = np.eye(128, dtype=np.float32)
    return A, negI, I128


def _build():
    import concourse.bass as bass
    import concourse.bacc as bacc
    import concourse.bass_isa as bass_isa
    import concourse.mybir as mybir
    import concourse.tile as tile
    from contextlib import ExitStack

    f32 = mybir.dt.float32
    bf16 = mybir.dt.bfloat16
    Alu = mybir.AluOpType

    nc = bacc.Bacc(
        "TRN2",
        target_bir_lowering=False,
        debug=False,
        num_devices=NCORES,
    )

    xs = nc.dram_tensor("xs", [B, D, XPP], f32, kind="ExternalInput")
    out = nc.dram_tensor("out", [B, D, OPP], f32, kind="ExternalOutput")
    A_np, negI_np, I_np = _consts()
    bandA_d = nc.inline_tensor(A_np, name="bandA")
    negI_d = nc.inline_tensor(negI_np, name="negI")
    ident_d = nc.inline_tensor(I_np, name="ident")

    # partition axis = (b, d) = 128; the (b, d) DRAM dims merge into a single
    # 128-partition dim during AP optimization since b's stride = 64 * d's.
    xsa = xs.ap()
    outa = out.ap()

    with ExitStack() as ctx:
        tc = ctx.enter_context(tile.TileContext(nc))
        pers = ctx.enter_context(tc.tile_pool(name="pers", bufs=1))
        maskp = ctx.enter_context(tc.tile_pool(name="mask", bufs=2))
        stagp = ctx.enter_context(tc.tile_pool(name="stag", bufs=2))
        psump = ctx.enter_context(tc.tile_pool(name="psum", bufs=4, space="PSUM"))
        dramp = ctx.enter_context(tc.tile_pool(name="dram", bufs=1, space="DRAM"))

        x_all = pers.tile([128, C, HA, W], f32)  # 136 KiB / partition
        pmax = pers.tile([128, 8], f32)
        pmin = pers.tile([128, 8], f32)
        red8 = pers.tile([128, 8], f32)  # per channel c: [mx_c, -mn_c] at 2c
        par8 = pers.tile([128, 8], f32)  # cross-partition reduced partials
        s1v = pers.tile([128, 8], f32)  # allreduced vals on partition 0
        gv8 = pers.tile([128, 8], f32)  # broadcast [mx, -mn] x 4 channels
        mnv = pers.tile([128, 4], f32)  # mn per channel
        h4 = pers.tile([128, 4], f32)  # 0.5*(mx-mn) per channel
        At = pers.tile([128, 128], bf16)
        Nt = pers.tile([128, 128], bf16)
        sel_bias = pers.tile([128, 1], f32)

        nc.vector.memset(sel_bias[:, :], -100.0)
        nc.sync.dma_start(out=At[:, :], in_=bandA_d.ap())
        nc.sync.dma_start(out=Nt[:, :], in_=negI_d.ap())

        # ---- bulk loads: 8 chunks of 2.2 MiB on the gpsimd SWDGE queue ----
        # FIFO ring order staggers completions so per-chunk reduces overlap.
        xv = x_all[:].rearrange("p c h w -> p (c h w)")
        for c in range(C):
            for hf in range(2):
                o = c * (HA * W) + hf * 17 * W
                nc.gpsimd.dma_start(
                    out=xv[:, o : o + 17 * W],
                    in_=xsa[:, :, o : o + 17 * W],
                )

        # ---- phase 1: min/max partials + 2 pipelined AllReduces ----
        # Per chunk: DVE takes the max pass; gpsimd (otherwise idle) folds the
        # min pass down to 1024 with two tensor_tensor mins, DVE finishes it.
        # Channels are grouped (0,1) and (2,3); each group gets its own
        # AllReduce so group-a thresholds land ~30us before group-b's and the
        # mask/dilate pipeline starts early.
        groups = [(0, 1), (2, 3)]
        for g, chans in enumerate(groups):
            for c in chans:
                for hf in range(2):
                    k = 2 * c + hf
                    # own rows of the chunk as a flat contiguous view
                    o1 = c * (HA * W) + (1 + 16 * hf) * W
                    rows = xv[:, o1 : o1 + 16 * W]
                    nc.vector.tensor_reduce(
                        out=pmax[:, k : k + 1],
                        in_=rows,
                        axis=mybir.AxisListType.X,
                        op=Alu.max,
                    )
                    nc.vector.tensor_reduce(
                        out=pmin[:, k : k + 1],
                        in_=rows,
                        axis=mybir.AxisListType.X,
                        op=Alu.min,
                    )
                nc.vector.tensor_reduce(
                    out=red8[:, 2 * c : 2 * c + 1],
                    in_=pmax[:, 2 * c : 2 * c + 2],
                    axis=mybir.AxisListType.X,
                    op=Alu.max,
                )
                nc.vector.tensor_reduce(
                    out=red8[:, 2 * c + 1 : 2 * c + 2],
                    in_=pmin[:, 2 * c : 2 * c + 2],
                    axis=mybir.AxisListType.X,
                    op=Alu.min,
                )
                # negate the min so a single AllReduce(max) covers both
                nc.vector.tensor_scalar_mul(
                    red8[:, 2 * c + 1 : 2 * c + 2],
                    red8[:, 2 * c + 1 : 2 * c + 2],
                    -1.0,
                )
            c0 = chans[0]
            # cross-partition max of the group's 4 partials on gpsimd, then
            # DMA out partition 0 and AllReduce(max) across the 8 cores.
            nc.gpsimd.partition_all_reduce(
                out_ap=par8[:, 2 * c0 : 2 * c0 + 4],
                in_ap=red8[:, 2 * c0 : 2 * c0 + 4],
                channels=128,
                reduce_op=bass_isa.ReduceOp.max,
            )
            cc_in = dramp.tile([1, 4], f32, tag=f"ccin{g}")
            cc_out = dramp.tile([1, 4], f32, tag=f"ccout{g}")
            nc.scalar.dma_start(
                out=cc_in[:, :], in_=par8[0:1, 2 * c0 : 2 * c0 + 4]
            )
            nc.gpsimd.collective_compute(
                "AllReduce",
                Alu.max,
                replica_groups=[list(range(NCORES))],
                ins=[cc_in[:, :].opt()],
                outs=[cc_out[:, :].opt()],
            )
            nc.sync.dma_start(
                out=s1v[0:1, 4 * g : 4 * g + 4],
                in_=cc_out[:, :],
            )

        # ---- phase 2: per group: thresholds, mask, dilate, boundary ----
        # both broadcasts emitted first: gpsimd's in-order program must not
        # park group b's broadcast behind group a's stores
        for g in range(2):
            nc.gpsimd.partition_broadcast(
                out_ap=gv8[:, 4 * g : 4 * g + 4],
                in_ap=s1v[0:1, 4 * g : 4 * g + 4],
            )
        for g, chans in enumerate(groups):
            for j, c in enumerate(chans):
                nc.vector.tensor_scalar_mul(
                    mnv[:, c : c + 1],
                    gv8[:, 4 * g + 2 * j + 1 : 4 * g + 2 * j + 2],
                    -1.0,
                )
                nc.vector.tensor_add(
                    h4[:, c : c + 1],
                    gv8[:, 4 * g + 2 * j : 4 * g + 2 * j + 1],
                    gv8[:, 4 * g + 2 * j + 1 : 4 * g + 2 * j + 2],
                )
                nc.vector.tensor_scalar_mul(h4[:, c : c + 1], h4[:, c : c + 1], 0.5)
            for c in chans:
                for hf in range(2):
                    # binm rows 0..17 = x halo rows 16*hf .. 16*hf+17
                    binm = maskp.tile([128, 18, W], bf16, tag="bin")
                    mH = maskp.tile([128, MHLEN], bf16, tag="mh")
                    if c == chans[0] and g == 0:
                        # zero the pad columns once per buffer (bufs=2 -> the
                        # first two iterations touch both buffers)
                        nc.vector.memset(mH[:, :], 0.0)
                    nc.vector.tensor_scalar(
                        out=binm[:, :, :],
                        in0=x_all[:, c, 16 * hf : 16 * hf + 18, :],
                        scalar1=mnv[:, c : c + 1],
                        scalar2=h4[:, c : c + 1],
                        op0=Alu.subtract,
                        op1=Alu.is_gt,
                    )
                    mHd = mH[:, MHW : MHW + 16 * MHW].rearrange(
                        "p (r z) -> p r z", z=MHW
                    )[:, :, 0:W]
                    nc.vector.tensor_tensor(
                        out=mHd,
                        in0=binm[:, 0:16, :],
                        in1=binm[:, 2:18, :],
                        op=Alu.max,
                    )
                    nc.vector.tensor_tensor(
                        out=mHd,
                        in0=mHd,
                        in1=binm[:, 1:17, :],
                        op=Alu.max,
                    )
                    for tp in range(2):  # 8 own rows per staging buffer
                        stg = stagp.tile([128, 2048], f32, tag="st")
                        for t in range(2):  # 4 own rows per PSUM tile
                            ps = psump.tile([128, 1024], f32, tag="ps")
                            for s in range(2):  # one PSUM bank = 2 rows
                                R = 8 * tp + 4 * t + 2 * s
                                pslice = ps[:, 512 * s : 512 * s + 512]
                                for jj, dw in enumerate((-1, 0, 1)):
                                    off = (R + 1) * MHW + dw
                                    rhs = mH[:, off : off + 2 * MHW].rearrange(
                                        "p (r z) -> p r z", z=MHW
                                    )[:, :, 0:W]
                                    nc.tensor.matmul(
                                        pslice,
                                        At[:, :],
                                        rhs,
                                        start=(jj == 0),
                                        stop=False,
                                    )
                                nc.tensor.matmul(
                                    pslice,
                                    Nt[:, :],
                                    binm[:, 1 + R : 3 + R, :],
                                    start=False,
                                    stop=True,
                                )
                            nc.scalar.activation(
                                out=stg[:, 1024 * t : 1024 * t + 1024],
                                in_=ps[:, :],
                                func=mybir.ActivationFunctionType.Sigmoid,
                                bias=sel_bias[:, :],
                                scale=200.0,
                            )
                        r0 = 16 * hf + 8 * tp  # own-row base in the shard
                        oo = c * (HS * W) + r0 * W
                        nc.gpsimd.dma_start(
                            out=outa[:, :, oo : oo + 8 * W],
                            in_=stg[:, :],
                        )

    nc.compile()
    return nc


def _get_nc():
    if "nc" not in _CACHE:
        _CACHE["nc"] = _build()
    return _CACHE["nc"]


def _make_in_maps(x: np.ndarray):
    # device shards use layout [B, D, C, HA, W] (flat per partition)
    xt = np.ascontiguousarray(x.transpose(0, 2, 1, 3, 4))  # [B, D, C, H, W]
    in_maps = []
    for k in range(NCORES):
        xs = np.empty((B, D, C, HA, W), np.float32)
        lo = k * HS
        xs[:, :, :, 1 : HS + 1, :] = xt[:, :, :, lo : lo + HS, :]
        if k > 0:
            xs[:, :, :, 0, :] = xt[:, :, :, lo - 1, :]
        else:
            xs[:, :, :, 0, :] = HPAD
        if k < NCORES - 1:
            xs[:, :, :, HS + 1, :] = xt[:, :, :, lo + HS, :]
        else:
            xs[:, :, :, HS + 1, :] = HPAD
        in_maps.append({"xs": xs.reshape(B, D, XPP)})
    return in_maps


def kernel(x: np.ndarray) -> np.ndarray:
    from concourse.bass_utils import run_bass_kernel_spmd

    x = np.ascontiguousarray(np.asarray(x), dtype=np.float32)
    assert x.shape == (B, C, D, H, W)

    in_maps = _make_in_maps(x)
    res = run_bass_kernel_spmd(_get_nc(), in_maps, core_ids=list(range(NCORES)))
    # shard outs are [B, D, C, HS, W]; back to [B, C, D, HS, W], concat on H
    pieces = [
        res.results[k]["out"].reshape(B, D, C, HS, W).transpose(0, 2, 1, 3, 4)
        for k in range(NCORES)
    ]
    return np.concatenate(pieces, axis=3)


if __name__ == "__main__":
    x = np.random.randn(B, C, D, H, W).astype(np.float32)
    y = kernel(x)
    print(y.shape, y.dtype, y.sum())
